# revision 6
# baseline (speedup 1.0000x reference)
"""Bass/Trainium2 kernel for nn_KbAttn (Bahdanau-style attention energies).

Math: out[b, l] = v . (W @ concat(h[b], k[l,b]) + bias)
Folding v into the weights (u1 = v@W1, u2 = v@W2, c = v.bias):
    out[b, l] = u2 . k[l, b, :] + (u1 . h[b] + c)
so the kernel is a pure memory-stream over k_embedding with a length-128
dot product per (l, b) — DMA-bound.

Sharding: data-parallel over B across 8 cores (256 rows each). The host
pre-transposes each k shard to [H, L, Bsh] (so per-partition DMA runs are
long and contiguous) and casts it to fp16 (halves HBM traffic; dot-product
absmax-relative error ~3e-4 with f32 PSUM accumulation). The PE computes
each dot-product column via matmul(psum[:, l], lhsT=kT_tile[h, b],
rhs=u2[h, 1]); bias s1c[b] is added on the DVE in f32 during PSUM->SBUF.
"""

import numpy as np

import concourse.bacc as bacc
import concourse.mybir as mybir
from concourse.tile import TileContext
from concourse.bass_utils import run_bass_kernel_spmd

M = 8            # cores
L = 431          # MAX_LEN
B = 2048
H = 128
BSH = B // M     # 256 batch rows per core
NL = 32          # l-slices per DMA chunk (2 MB fp16 per chunk)

FP32 = mybir.dt.float32
FP16 = mybir.dt.float16


def _build_nc():
    nc = bacc.Bacc()
    kt = nc.dram_tensor("kt", [H, L, BSH], FP16, kind="ExternalInput")
    u2 = nc.dram_tensor("u2", [H, 1], FP16, kind="ExternalInput")
    s1c = nc.dram_tensor("s1c", [2, H, 1], FP32, kind="ExternalInput")
    out = nc.dram_tensor("out", [BSH, L], FP32, kind="ExternalOutput")

    with TileContext(nc) as tc:
        with (
            tc.tile_pool(name="const", bufs=1) as cpool,
            tc.tile_pool(name="kbuf", bufs=3) as kpool,
            tc.tile_pool(name="obuf", bufs=1) as opool,
            tc.tile_pool(name="psum", bufs=1, space="PSUM") as ppool,
        ):
            u2_t = cpool.tile([H, 1], FP16, tag="u2", name="u2t")
            nc.gpsimd.dma_start(out=u2_t[:], in_=u2[:])
            s1c_t = []
            for bh in range(2):
                t = cpool.tile([H, 1], FP32, tag=f"s1c{bh}", name=f"s1ct{bh}")
                nc.gpsimd.dma_start(out=t[:], in_=s1c[bh])
                s1c_t.append(t)

            psum_t = [
                ppool.tile([H, 512], FP32, tag=f"ps{bh}", name=f"ps{bh}")
                for bh in range(2)
            ]

            for l0 in range(0, L, NL):
                nl = min(NL, L - l0)
                ktile = kpool.tile([H, NL, BSH], FP16, tag="k", name="ktile")
                nc.sync.dma_start(
                    out=ktile[:, :nl, :], in_=kt[:, l0 : l0 + nl, :]
                )
                for i in range(nl):
                    for bh in range(2):
                        nc.tensor.matmul(
                            psum_t[bh][:, l0 + i : l0 + i + 1],
                            lhsT=ktile[:, i, bh * H : (bh + 1) * H],
                            rhs=u2_t[:],
                            start=True,
                            stop=True,
                        )

            for bh in range(2):
                o_t = opool.tile([H, L], FP32, tag=f"o{bh}", name=f"ot{bh}")
                nc.vector.tensor_scalar_add(
                    out=o_t[:], in0=psum_t[bh][:, :L], scalar1=s1c_t[bh][:]
                )
                nc.sync.dma_start(out=out[bh * H : (bh + 1) * H, :], in_=o_t[:])
    nc.compile()
    return nc


def _prep_in_maps(hidden, k_embedding, attn_w, attn_b, v):
    hidden = np.asarray(hidden, dtype=np.float32)
    k_embedding = np.asarray(k_embedding, dtype=np.float32)
    attn_w = np.asarray(attn_w, dtype=np.float32)
    attn_b = np.asarray(attn_b, dtype=np.float32)
    v = np.asarray(v, dtype=np.float32)

    u = v[0] @ attn_w                       # [2H]
    u1, u2 = u[:H], u[H:]
    c = float(v[0] @ attn_b)
    s1c = hidden[0] @ u1 + c                # [B]

    u2_col = np.ascontiguousarray(u2.reshape(H, 1)).astype(np.float16)
    k16 = k_embedding.astype(np.float16)    # cast once, then per-shard transpose
    in_maps = []
    for m in range(M):
        ksh = np.ascontiguousarray(
            k16[:, m * BSH : (m + 1) * BSH, :].transpose(2, 0, 1)
        )                                    # [H, L, BSH] fp16
        in_maps.append(
            {
                "kt": ksh,
                "u2": u2_col,
                "s1c": np.ascontiguousarray(
                    s1c[m * BSH : (m + 1) * BSH].reshape(2, H, 1)
                ),
            }
        )
    return in_maps


def _run(inputs, **spmd_kwargs):
    nc = _build_nc()
    in_maps = _prep_in_maps(**inputs)
    res = run_bass_kernel_spmd(nc, in_maps, list(range(M)), **spmd_kwargs)
    out = np.concatenate([res.results[m]["out"] for m in range(M)], axis=0)
    return out, res


def kernel(**inputs) -> np.ndarray:
    out, _ = _run(inputs)
    return out


# revision 7
# speedup vs baseline: 1.0050x; 1.0050x over previous
"""Bass/Trainium2 kernel for nn_KbAttn (Bahdanau-style attention energies).

Math: out[b, l] = v . (W @ concat(h[b], k[l,b]) + bias)
Folding v into the weights (u1 = v@W1, u2 = v@W2, c = v.bias):
    out[b, l] = u2 . k[l, b, :] + (u1 . h[b] + c)
so the kernel is a pure memory-stream over k_embedding with a length-128
dot product per (l, b) — DMA-bound.

Sharding: data-parallel over B across 8 cores (256 rows each). The host
pre-transposes each k shard to [H, L, Bsh] (so per-partition DMA runs are
long and contiguous) and casts it to fp16 (halves HBM traffic; dot-product
absmax-relative error ~3e-4 with f32 PSUM accumulation). The PE computes
each dot-product column via matmul(psum[:, l], lhsT=kT_tile[h, b],
rhs=u2[h, 1]); bias s1c[b] is added on the DVE in f32 during PSUM->SBUF.
"""

import numpy as np

import concourse.bacc as bacc
import concourse.mybir as mybir
from concourse.tile import TileContext
from concourse.bass_utils import run_bass_kernel_spmd

M = 8            # cores
L = 431          # MAX_LEN
B = 2048
H = 128
BSH = B // M     # 256 batch rows per core
NL = 32          # l-slices per DMA chunk (2 MB fp16 per chunk)

FP32 = mybir.dt.float32
FP16 = mybir.dt.float16


def _build_nc():
    nc = bacc.Bacc()
    kt = nc.dram_tensor("kt", [H, L, BSH], FP16, kind="ExternalInput")
    u2 = nc.dram_tensor("u2", [H, 1], FP16, kind="ExternalInput")
    s1c = nc.dram_tensor("s1c", [2, H, 1], FP32, kind="ExternalInput")
    out = nc.dram_tensor("out", [BSH, L], FP32, kind="ExternalOutput")

    with TileContext(nc) as tc:
        with (
            tc.tile_pool(name="const", bufs=1) as cpool,
            tc.tile_pool(name="kbuf", bufs=3) as kpool,
            tc.tile_pool(name="obuf", bufs=1) as opool,
            tc.tile_pool(name="psum", bufs=1, space="PSUM") as ppool,
        ):
            u2_t = cpool.tile([H, 1], FP16, tag="u2", name="u2t")
            nc.gpsimd.dma_start(out=u2_t[:], in_=u2[:])
            s1c_t = []
            for bh in range(2):
                t = cpool.tile([H, 1], FP32, tag=f"s1c{bh}", name=f"s1ct{bh}")
                nc.gpsimd.dma_start(out=t[:], in_=s1c[bh])
                s1c_t.append(t)

            psum_t = [
                ppool.tile([H, 512], FP32, tag=f"ps{bh}", name=f"ps{bh}")
                for bh in range(2)
            ]
            o_t = [
                opool.tile([H, L], FP32, tag=f"o{bh}", name=f"ot{bh}")
                for bh in range(2)
            ]

            chunks = [(l0, min(NL, L - l0)) for l0 in range(0, L, NL)]
            last_l0 = chunks[-1][0]
            for l0, nl in chunks:
                ktile = kpool.tile([H, NL, BSH], FP16, tag="k", name="ktile")
                nc.sync.dma_start(
                    out=ktile[:, :nl, :], in_=kt[:, l0 : l0 + nl, :]
                )
                for i in range(nl):
                    for bh in range(2):
                        nc.tensor.matmul(
                            psum_t[bh][:, l0 + i : l0 + i + 1],
                            lhsT=ktile[:, i, bh * H : (bh + 1) * H],
                            rhs=u2_t[:],
                            start=True,
                            stop=True,
                        )
                if l0 + nl == last_l0:
                    # flush cols [0, last_l0) now — the big PSUM->SBUF+bias op
                    # overlaps the final chunk's matmuls
                    for bh in range(2):
                        nc.vector.tensor_scalar_add(
                            out=o_t[bh][:, :last_l0],
                            in0=psum_t[bh][:, :last_l0],
                            scalar1=s1c_t[bh][:],
                        )

            for bh in range(2):
                nc.vector.tensor_scalar_add(
                    out=o_t[bh][:, last_l0:],
                    in0=psum_t[bh][:, last_l0:L],
                    scalar1=s1c_t[bh][:],
                )
                nc.sync.dma_start(out=out[bh * H : (bh + 1) * H, :], in_=o_t[bh][:])
    nc.compile()
    return nc


def _prep_in_maps(hidden, k_embedding, attn_w, attn_b, v):
    hidden = np.asarray(hidden, dtype=np.float32)
    k_embedding = np.asarray(k_embedding, dtype=np.float32)
    attn_w = np.asarray(attn_w, dtype=np.float32)
    attn_b = np.asarray(attn_b, dtype=np.float32)
    v = np.asarray(v, dtype=np.float32)

    u = v[0] @ attn_w                       # [2H]
    u1, u2 = u[:H], u[H:]
    c = float(v[0] @ attn_b)
    s1c = hidden[0] @ u1 + c                # [B]

    u2_col = np.ascontiguousarray(u2.reshape(H, 1)).astype(np.float16)
    k16 = k_embedding.astype(np.float16)    # cast once, then per-shard transpose
    in_maps = []
    for m in range(M):
        ksh = np.ascontiguousarray(
            k16[:, m * BSH : (m + 1) * BSH, :].transpose(2, 0, 1)
        )                                    # [H, L, BSH] fp16
        in_maps.append(
            {
                "kt": ksh,
                "u2": u2_col,
                "s1c": np.ascontiguousarray(
                    s1c[m * BSH : (m + 1) * BSH].reshape(2, H, 1)
                ),
            }
        )
    return in_maps


def _run(inputs, **spmd_kwargs):
    nc = _build_nc()
    in_maps = _prep_in_maps(**inputs)
    res = run_bass_kernel_spmd(nc, in_maps, list(range(M)), **spmd_kwargs)
    out = np.concatenate([res.results[m]["out"] for m in range(M)], axis=0)
    return out, res


def kernel(**inputs) -> np.ndarray:
    out, _ = _run(inputs)
    return out
